# revision 4
# baseline (speedup 1.0000x reference)
"""Multi-head attention (B=2, S=2048, D=1024, H=16) on 8 Trainium2 NeuronCores.

Sharding: data-parallel on batch, tensor-parallel on heads.
Core c handles batch b = c // 4 and heads [4*(c%4), 4*(c%4)+4).
Each core computes its 4 heads' attention + its partial Wo projection;
the host sums the 4 partial [S, D] outputs per batch (the TP all-reduce).

Device-side layout choices (see comments inline):
- Host pre-transposes query/context to [D, S] (bf16) so every matmul
  contraction dim lands on SBUF partitions with no on-device transposes.
- Scores are computed transposed ([c, q]) so the PV matmul consumes the
  exp'd probabilities directly as the moving operand, and the softmax
  denominator comes free as a 65th row of the PV output (ones column in V).
- bf16 storage/matmul operands, fp32 PSUM accumulation throughout.
"""

import numpy as np
import ml_dtypes

import concourse.bacc as bacc
import concourse.mybir as mybir
from concourse.tile import TileContext
from concourse.bass_utils import run_bass_kernel_spmd

BF16 = mybir.dt.bfloat16
F32 = mybir.dt.float32

B, S, D, H = 2, 2048, 1024, 16
SPH = D // H          # 64
NH = 4                # heads per core
P = 128               # SBUF partitions
DC = D // P           # 8 d-chunks
CT = S // P           # 16 c-tiles
QT = S // P           # 16 q-tiles
NEG_INF = -1e9

_NC_CACHE = {}


def _build(masked: bool):
    nc = bacc.Bacc("TRN2", target_bir_lowering=False, debug=False, num_devices=8)

    qt_d = nc.declare_dram_parameter("qt", [D, S], BF16, isOutput=False)
    ct_d = nc.declare_dram_parameter("ctx", [D, S], BF16, isOutput=False)
    wq_d = nc.declare_dram_parameter("wq", [D, NH * SPH], BF16, isOutput=False)
    wk_d = nc.declare_dram_parameter("wk", [D, NH * SPH], BF16, isOutput=False)
    wv_d = nc.declare_dram_parameter("wv", [D, NH * SPH], BF16, isOutput=False)
    wo_d = nc.declare_dram_parameter("wo", [NH * SPH, D], BF16, isOutput=False)
    if masked:
        mk_d = nc.declare_dram_parameter("maskT", [S, S], BF16, isOutput=False)
    out_d = nc.declare_dram_parameter("out", [S, D], F32, isOutput=True)

    with TileContext(nc) as tc:
        with (
            tc.tile_pool(name="const", bufs=1) as const,
            tc.tile_pool(name="work", bufs=1) as work,
            tc.tile_pool(name="pt", bufs=3) as ptp,
            tc.tile_pool(name="outp", bufs=2) as outp,
            tc.tile_pool(name="psS", bufs=2, space="PSUM") as psS,
            tc.tile_pool(name="psA", bufs=2, space="PSUM") as psA,
        ):
            # ---- stage inputs in SBUF -------------------------------------
            qt_sb = const.tile([P, DC, S], BF16)
            nc.sync.dma_start(out=qt_sb, in_=qt_d[:, :].rearrange("(c p) q -> p c q", p=P))
            ct_sb = const.tile([P, DC, S], BF16)
            nc.sync.dma_start(out=ct_sb, in_=ct_d[:, :].rearrange("(c p) q -> p c q", p=P))
            wq_sb = const.tile([P, DC, NH * SPH], BF16)
            nc.sync.dma_start(out=wq_sb, in_=wq_d[:, :].rearrange("(c p) n -> p c n", p=P))
            wk_sb = const.tile([P, DC, NH * SPH], BF16)
            nc.sync.dma_start(out=wk_sb, in_=wk_d[:, :].rearrange("(c p) n -> p c n", p=P))
            wv_sb = const.tile([P, DC, NH * SPH], BF16)
            nc.sync.dma_start(out=wv_sb, in_=wv_d[:, :].rearrange("(c p) n -> p c n", p=P))
            # wo rows are (h, s); head pair t = h//2 packs two heads into the
            # partition dim (head h%2==0 -> partitions 0-63, ==1 -> 64-127).
            wo_sb = const.tile([P, 2, D], BF16)
            nc.sync.dma_start(out=wo_sb, in_=wo_d[:, :].rearrange("(t x) d -> x t d", x=P))

            # ---- projections: qT/kT [64, S] per head, packed per pair -----
            qTp = [work.tile([P, S], BF16, tag=f"qT{p}", name=f"qT{p}") for p in range(2)]
            kTp = [work.tile([P, S], BF16, tag=f"kT{p}", name=f"kT{p}") for p in range(2)]
            for p in range(2):
                for src_sb, dst in ((wq_sb, qTp[p]), (wk_sb, kTp[p])):
                    for qc4 in range(4):
                        ps = psS.tile([P, 512], F32, tag="S")
                        for dc in range(DC):
                            nc.tensor.matmul(
                                ps[:, :],
                                src_sb[:, dc, P * p:P * (p + 1)],
                                qt_sb[:, dc, 512 * qc4:512 * (qc4 + 1)] if src_sb is wq_sb
                                else ct_sb[:, dc, 512 * qc4:512 * (qc4 + 1)],
                                start=(dc == 0), stop=(dc == DC - 1),
                            )
                        nc.scalar.copy(dst[:, 512 * qc4:512 * (qc4 + 1)], ps[:, :])

            # ---- V in natural [c, (h, s)] layout + ones column ------------
            vaug = work.tile([P, CT, NH, SPH + 1], BF16)
            nc.vector.memset(vaug[:, :, :, SPH:SPH + 1], 1.0)
            for ct in range(CT):
                psv = psS.tile([P, NH * SPH], F32, tag="S")
                for dc in range(DC):
                    nc.tensor.matmul(
                        psv[:, :],
                        ct_sb[:, dc, P * ct:P * (ct + 1)],
                        wv_sb[:, dc, :],
                        start=(dc == 0), stop=(dc == DC - 1),
                    )
                nc.vector.tensor_copy(
                    vaug[:, ct, :, 0:SPH],
                    psv[:, :].rearrange("p (h s) -> p h s", h=NH),
                )

            # ---- attention per head ---------------------------------------
            # outT_qc[qc]: [(a*64+s) partition, pair, 1024 q] bf16
            outT_qc = [work.tile([P, 2, 1024], BF16, tag=f"oT{qc}", name=f"oT{qc}") for qc in range(2)]
            for h in range(NH):
                p, a = h // 2, h % 2
                lo, hi = SPH * a, SPH * (a + 1)
                kTh = kTp[p][lo:hi, :]
                qTh = qTp[p][lo:hi, :]
                for qc in range(2):
                    q0 = 1024 * qc
                    po = psA.tile([SPH + 1, 1024], F32, tag="A")
                    # 1-deep software pipeline: scores(ct) ahead of PV(ct-1)
                    pend = None  # (Sps, PT-ready marker) from previous ct
                    for ct in range(CT + 1):
                        if ct < CT:
                            Sps = psS.tile([P, 1024], F32, tag="S")
                            nc.tensor.matmul(
                                Sps[:, 0:512], kTh[:, P * ct:P * (ct + 1)],
                                qTh[:, q0:q0 + 512], start=True, stop=True)
                            nc.tensor.matmul(
                                Sps[:, 512:1024], kTh[:, P * ct:P * (ct + 1)],
                                qTh[:, q0 + 512:q0 + 1024], start=True, stop=True)
                            if masked:
                                mk = ptp.tile([P, 1024], BF16, tag="mk")
                                nc.sync.dma_start(
                                    out=mk,
                                    in_=mk_d[P * ct:P * (ct + 1), q0:q0 + 1024])
                                nc.vector.tensor_add(Sps[:, :], Sps[:, :], mk)
                        if ct >= 1:
                            pct = ct - 1
                            pSps = pend
                            PT = ptp.tile([P, 1024], BF16, tag="PT")
                            nc.scalar.activation(
                                PT, pSps, mybir.ActivationFunctionType.Exp)
                            nc.tensor.matmul(
                                po[:, 0:512], vaug[:, pct, h, :], PT[:, 0:512],
                                start=(pct == 0), stop=(pct == CT - 1))
                            nc.tensor.matmul(
                                po[:, 512:1024], vaug[:, pct, h, :], PT[:, 512:1024],
                                start=(pct == 0), stop=(pct == CT - 1))
                        if ct < CT:
                            pend = Sps
                    # epilogue: normalize rows 0-63 by reciprocal of row 64.
                    # Engines cannot shift partitions, so the denominator row
                    # (partition 64) goes PSUM -> SBUF (DVE, base-matched),
                    # then SBUF row 64 -> row 0 via DMA, then gpsimd broadcast
                    # (which only reads partition 0 correctly on HW).
                    srow = outp.tile([P, 1024], F32, tag="srow", name="srow")
                    nc.vector.tensor_copy(srow[SPH:SPH + 1, :], po[SPH:SPH + 1, :])
                    drow = outp.tile([1, 1024], F32, tag="drow", name="drow")
                    nc.sync.dma_start(out=drow[0:1, :], in_=srow[SPH:SPH + 1, :])
                    rb = outp.tile([SPH, 1024], F32, tag="rb", name="rb")
                    nc.gpsimd.partition_broadcast(rb, drow[0:1, :], channels=SPH)
                    rb2 = outp.tile([SPH, 1024], F32, tag="rb2", name="rb2")
                    nc.vector.reciprocal_approx_fast(rb2, rb)
                    if a == 0:
                        nc.vector.tensor_mul(outT_qc[qc][0:SPH, p, :], po[0:SPH, :], rb2)
                    else:
                        ot = ptp.tile([SPH, 1024], BF16, tag="ott", name="ot")
                        nc.vector.tensor_mul(ot, po[0:SPH, :], rb2)
                        # partition shift 0-63 -> 64-127 has to go through DMA
                        nc.sync.dma_start(out=outT_qc[qc][SPH:P, p, :], in_=ot)

            # ---- output projection --------------------------------------
            # Concurrent row-group matmuls (head pairs packed at partition 0
            # and 64) may not accumulate into the same PSUM bank (HW hang), so
            # each row group gets its own accumulator; DVE adds them.
            for qt in range(QT):
                qc, off = qt // 8, (qt % 8) * P
                pool = psA if qt % 2 == 0 else psS
                tagw = "A" if qt % 2 == 0 else "S"
                wops0 = pool.tile([P, D], F32, tag=tagw, name="wops0")
                wops1 = pool.tile([P, D], F32, tag=tagw, name="wops1")
                for p in range(2):
                    for a in range(2):
                        lo, hi = SPH * a, SPH * (a + 1)
                        wx = wops0 if a == 0 else wops1
                        for dh in range(2):
                            nc.tensor.matmul(
                                wx[:, 512 * dh:512 * (dh + 1)],
                                outT_qc[qc][lo:hi, p, off:off + P],
                                wo_sb[lo:hi, p, 512 * dh:512 * (dh + 1)],
                                start=(p == 0),
                                stop=(p == 1),
                            )
                tcp = outp.tile([P, D], F32, tag="tcp", name="tcp")
                nc.scalar.copy(tcp, wops1)
                osb = outp.tile([P, D], F32, tag="osb", name="osb")
                nc.vector.tensor_add(osb, wops0, tcp)
                nc.sync.dma_start(out=out_d[P * qt:P * (qt + 1), :], in_=osb)

    nc.compile()
    return nc


def _get_nc(masked: bool):
    if masked not in _NC_CACHE:
        _NC_CACHE[masked] = _build(masked)
    return _NC_CACHE[masked]


def kernel(query, context, attention_mask, Wq, Wk, Wv, Wo, **_unused):
    query = np.asarray(query, dtype=np.float32)
    context = np.asarray(context, dtype=np.float32)
    attention_mask = np.asarray(attention_mask, dtype=np.float32)
    Wq = np.asarray(Wq, dtype=np.float32)
    Wk = np.asarray(Wk, dtype=np.float32)
    Wv = np.asarray(Wv, dtype=np.float32)
    Wo = np.asarray(Wo, dtype=np.float32)

    masked = bool(np.any(attention_mask))
    nc = _get_nc(masked)

    bf = ml_dtypes.bfloat16
    # fold the 1/sqrt(SPH) score scale into Wq
    wq_s = (Wq * (SPH ** -0.5)).astype(bf)
    wk_s = Wk.astype(bf)
    wv_s = Wv.astype(bf)
    wo_s = Wo.astype(bf)

    qtT = [np.ascontiguousarray(query[b].T).astype(bf) for b in range(B)]
    ctT = [np.ascontiguousarray(context[b].T).astype(bf) for b in range(B)]
    if masked:
        mkT = [np.ascontiguousarray((attention_mask[b, 0] * NEG_INF).T).astype(bf)
               for b in range(B)]

    in_maps = []
    for c in range(8):
        b, g = c // 4, c % 4
        hs = slice(NH * g, NH * (g + 1))
        im = {
            "qt": qtT[b],
            "ctx": ctT[b],
            "wq": np.ascontiguousarray(wq_s[:, hs, :]).reshape(D, NH * SPH),
            "wk": np.ascontiguousarray(wk_s[:, hs, :]).reshape(D, NH * SPH),
            "wv": np.ascontiguousarray(wv_s[:, hs, :]).reshape(D, NH * SPH),
            "wo": np.ascontiguousarray(wo_s[hs]).reshape(NH * SPH, D),
        }
        if masked:
            im["maskT"] = mkT[b]
        in_maps.append(im)

    res = run_bass_kernel_spmd(nc, in_maps, core_ids=list(range(8)))

    out = np.zeros((B, S, D), dtype=np.float32)
    for c in range(8):
        out[c // 4] += res.results[c]["out"]
    return out
